# revision 2
# baseline (speedup 1.0000x reference)
"""Trainium2 Bass kernel for a cross-attention block with 3D-coordinate RoPE.

Module: q/k/v projections of x [B,Tq,D] against memory [B,Tk,D], 3D-coord
rotary embedding on q/k, softmax(q k^T / sqrt(Hd)) v, output projection.
B=2, Tq=1024, Tk=2048, D=1536, 16 heads x 96.

Sharding: 8 cores = (2 batches) x (4 head-groups of 4 heads). Each core
computes its heads end-to-end plus a partial output projection; the host
sums the 4 partials per batch. Biases bv/bo are folded in on the host
(attention rows sum to one), bq/bk are added on-device during PSUM
eviction.

Layout: feature-major ("transposed") on device. Scores are computed
transposed (S^T = k q^T) so the PV matmul needs no on-chip transposes;
softmax denominators come from a ones-column appended to v; the
per-query normalization is broadcast across partitions with a K=1
ones-vector matmul of the reciprocal row.

Schedule notes (v2):
- memory^T streams from HBM twice (K phase and V phase), paced by its
  consumers, so the Q phase is compute-bound instead of fighting a 6MB
  resident preload for HBM bandwidth.
- the RoPE half-rotation swap is two SBUF->SBUF DMA copies per tile
  (partition remap) instead of a PE permutation matmul; the combine is
  three in-place DVE ops. TensorE runs zero RoPE columns.
- per-head softmax normalization takes the reciprocal of the [1,Tq]
  denominator row first, then broadcasts it with the ones-matmul, so the
  post-PV critical path is two tiny DVE ops + one 1024-col matmul.
- O-projection accumulates in a dedicated 4-bank PSUM pool (two m-tiles
  in flight).

Matmul dtype is selectable via KMM_DTYPE in {bf16, f16, f32r, f32};
logits, softmax and denominators stay fp32 in all modes.
"""

import os
import sys

sys.path.insert(0, "/opt/trn_rl_repo")

import numpy as np
import ml_dtypes
from contextlib import ExitStack

import concourse.bass as bass
import concourse.tile as tile
from concourse import bacc, mybir
from concourse.bass_utils import run_bass_kernel_spmd

# ---------------------------------------------------------------- constants
B = 2
TQ = 1024
TK = 2048
D = 1536
NH = 16
HD = 96
ROPE_HALF = HD // 2           # 48
FREQ_PER_AXIS = ROPE_HALF // 3  # 16
ROPE_BASE = 10000.0
NH_CORE = 4                   # heads per core
HG = NH_CORE * HD             # 384 features per core
KC = D // 128                 # 12 contraction chunks
MTILES = D // 128             # 12 output-row tiles of the o-projection
SCALE = 1.0 / float(np.sqrt(HD))
N_CORES = 8
VW = HD + 1                   # 97: head-dim + ones column

F32 = mybir.dt.float32

_MM_DT_NAME = os.environ.get("KMM_DTYPE", "f16")
_DT = {"f32r": mybir.dt.float32r, "f32": mybir.dt.float32,
       "bf16": mybir.dt.bfloat16, "f16": mybir.dt.float16}
_NP = {"f32r": np.float32, "f32": np.float32, "bf16": ml_dtypes.bfloat16,
       "f16": np.float16}
if _MM_DT_NAME == "mixed":          # q/k chain fp32r, v/attn-weight/out bf16
    _QK_NAME, _PV_NAME = "f32r", "bf16"
elif _MM_DT_NAME == "mixed16":      # q/k chain fp32r, rest fp16
    _QK_NAME, _PV_NAME = "f32r", "f16"
else:
    _QK_NAME = _PV_NAME = _MM_DT_NAME
QK_DT, QK_NP = _DT[_QK_NAME], _NP[_QK_NAME]
PV_DT, PV_NP = _DT[_PV_NAME], _NP[_PV_NAME]
SPLIT_MEM = _QK_NAME != _PV_NAME    # ship memory twice (per-dtype) if mixed


# ---------------------------------------------------------------- bass build
def _build_nc():
    nc = bacc.Bacc(trn_type="TRN2", target_bir_lowering=False, debug=False)

    io = {}
    def dram_in(name, shape, dt):
        io[name] = nc.dram_tensor(name, list(shape), dt, kind="ExternalInput").ap()
    dram_in("xT", [D, TQ], QK_DT)
    dram_in("memT", [D, TK], QK_DT)
    if SPLIT_MEM:
        dram_in("memTv", [D, TK], PV_DT)
    dram_in("wqT", [D, HG], QK_DT)  # columns of Wq^T for this head group
    dram_in("wkT", [D, HG], QK_DT)
    dram_in("wvT", [D, HG], PV_DT)
    dram_in("woT", [HG, D], PV_DT)  # rows of Wo^T for this head group
    dram_in("bq4", [HD, NH_CORE], F32)
    dram_in("bk4", [HD, NH_CORE], F32)
    dram_in("cqE", [HD, TQ], F32)  # cos table, feature-major, q side
    dram_in("sqE", [HD, TQ], F32)  # sign-folded sin table, q side
    dram_in("ckE", [HD, TK], F32)
    dram_in("skE", [HD, TK], F32)
    dram_in("ones1", [1, 128], PV_DT)
    dram_in("ones4", [128, NH_CORE], PV_DT)
    oT = nc.dram_tensor("oT", [D, TQ], F32, kind="ExternalOutput").ap()

    with tile.TileContext(nc) as tc, ExitStack() as ctx:
        _body(ctx, tc, io, oT)
    nc.compile()
    return nc


def _body(ctx, tc, io, oT):
    nc = tc.nc
    P = 128
    NKC = TK // P
    Exp = mybir.ActivationFunctionType.Exp
    Ident = mybir.ActivationFunctionType.Identity

    const = ctx.enter_context(tc.tile_pool(name="const", bufs=1))
    resident = ctx.enter_context(tc.tile_pool(name="resident", bufs=1))

    ones1_t = const.tile([1, P], PV_DT, name="ones1_t")
    bq_t = const.tile([HD, NH_CORE], F32, name="bq_t")
    bk_t = const.tile([HD, NH_CORE], F32, name="bk_t")
    cq_t = const.tile([HD, TQ], F32, name="cq_t")
    sq_t = const.tile([HD, TQ], F32, name="sq_t")
    ck_t = const.tile([HD, TK], F32, name="ck_t")
    sk_t = const.tile([HD, TK], F32, name="sk_t")
    wk_all = const.tile([P, KC, HG], QK_DT, name="wk_all")
    wv_all = const.tile([P, KC, HG], PV_DT, name="wv_all")
    qT = [resident.tile([HD, TQ], QK_DT, name=f"qT{h}", tag=f"qT{h}")
          for h in range(NH_CORE)]
    kT = [resident.tile([HD, TK], QK_DT, name=f"kT{h}", tag=f"kT{h}")
          for h in range(NH_CORE)]
    qsw = [resident.tile([HD, TQ], QK_DT, name=f"qsw{h}", tag=f"qsw{h}")
           for h in range(NH_CORE)]
    ksw = [resident.tile([HD, TK], QK_DT, name=f"ksw{h}", tag=f"ksw{h}")
           for h in range(NH_CORE)]
    vst = [resident.tile([P, NH_CORE * VW], PV_DT, name=f"vst{m}", tag=f"vst{m}")
           for m in range(NKC)]
    tmp_pool = ctx.enter_context(tc.tile_pool(name="tmp_pool", bufs=2))

    # ---- phase Q: q^T = Wq_h @ x^T (c-outer, 8 psum banks) ---------------
    with ExitStack() as qctx:
        psq_pool = qctx.enter_context(
            tc.tile_pool(name="psq", bufs=NH_CORE, space="PSUM"))
        xq_pool = qctx.enter_context(tc.tile_pool(name="xq", bufs=4))
        wq_pool = qctx.enter_context(tc.tile_pool(name="wq", bufs=1))
        wq_all = wq_pool.tile([P, KC, HG], QK_DT, name="wq_all")
        psq = [psq_pool.tile([HD, TQ], F32, name=f"psq{h}", tag="psq")
               for h in range(NH_CORE)]
        for c in range(KC):
            nc.sync.dma_start(out=wq_all[:, c, :],
                              in_=io["wqT"][c * P:(c + 1) * P, :])
            xc = xq_pool.tile([P, TQ], QK_DT, name="xc", tag="xc")
            nc.sync.dma_start(out=xc[:, 0:512],
                              in_=io["xT"][c * P:(c + 1) * P, 0:512])
            nc.sync.dma_start(out=xc[:, 512:1024],
                              in_=io["xT"][c * P:(c + 1) * P, 512:1024])
            nc.sync.dma_start(out=wk_all[:, c, :],
                              in_=io["wkT"][c * P:(c + 1) * P, :])

            for h in range(NH_CORE):
                lhs = wq_all[:, c, h * HD:(h + 1) * HD]
                for n in range(2):
                    nc.tensor.matmul(
                        psq[h][:, n * 512:(n + 1) * 512],
                        lhs, xc[:, n * 512:(n + 1) * 512],
                        start=(c == 0), stop=(c == KC - 1))
        nc.sync.dma_start(out=bq_t[:], in_=io["bq4"][:])
        for h in range(NH_CORE):
            if h % 2 == 0:
                nc.vector.tensor_scalar_add(qT[h][:], psq[h][:],
                                            bq_t[:, h:h + 1])
            else:
                nc.scalar.activation(qT[h][:], psq[h][:], Ident,
                                     bias=bq_t[:, h:h + 1])

    # constants needed next (emitted after Q's loads so Q starts sooner)
    nc.sync.dma_start(out=cq_t[:], in_=io["cqE"][:])
    nc.sync.dma_start(out=sq_t[:], in_=io["sqE"][:])
    nc.sync.dma_start(out=bk_t[:], in_=io["bk4"][:])
    nc.sync.dma_start(out=ck_t[:], in_=io["ckE"][:])
    nc.sync.dma_start(out=sk_t[:], in_=io["skE"][:])
    nc.sync.dma_start(out=ones1_t[:], in_=io["ones1"][:])

    def rope_dma(dst, sw, cE, sE, lo, width):
        """Rotate dst[:, lo:lo+width] in place: swap halves via SBUF->SBUF
        DMA into sw, then dst = dst*cE + sw*sE (three in-place DVE ops)."""
        sl = slice(lo, lo + width)
        nc.sync.dma_start(out=sw[0:ROPE_HALF, sl],
                          in_=dst[ROPE_HALF:HD, sl])
        nc.sync.dma_start(out=sw[ROPE_HALF:HD, sl],
                          in_=dst[0:ROPE_HALF, sl])
        nc.vector.tensor_mul(dst[:, sl], dst[:, sl], cE[:, sl])
        nc.vector.tensor_mul(sw[:, sl], sw[:, sl], sE[:, sl])
        nc.vector.tensor_add(dst[:, sl], dst[:, sl], sw[:, sl])

    # rope q: runs on DMA+DVE entirely, overlapping the K matmuls
    for h in range(NH_CORE):
        rope_dma(qT[h], qsw[h], cq_t, sq_t, 0, TQ)

    # ---- phase K: k^T = Wk_h @ mem^T (c-outer, by Tk quarter) ------------
    with ExitStack() as kctx:
        psk_pool = kctx.enter_context(
            tc.tile_pool(name="psk", bufs=8, space="PSUM"))
        mh_pool = kctx.enter_context(tc.tile_pool(name="mhk", bufs=16))
        for q4 in range(4):
            col = q4 * 512
            psk = [psk_pool.tile([HD, 512], F32, name=f"psk{q4}_{h}", tag="psk")
                   for h in range(NH_CORE)]
            for c in range(KC):
                mct = mh_pool.tile([P, 512], QK_DT, name="mck", tag="mck")
                nc.sync.dma_start(
                    out=mct[:],
                    in_=io["memT"][c * P:(c + 1) * P, col:col + 512])
                for h in range(NH_CORE):
                    nc.tensor.matmul(
                        psk[h][:], wk_all[:, c, h * HD:(h + 1) * HD], mct[:],
                        start=(c == 0), stop=(c == KC - 1))
            for h in range(NH_CORE):
                if h % 2 == 0:
                    nc.vector.tensor_scalar_add(kT[h][:, col:col + 512],
                                                psk[h][:], bk_t[:, h:h + 1])
                else:
                    nc.scalar.activation(kT[h][:, col:col + 512],
                                         psk[h][:], Ident,
                                         bias=bk_t[:, h:h + 1])
            for h in range(NH_CORE):
                rope_dma(kT[h], ksw[h], ck_t, sk_t, col, 512)
            if q4 == 1:
                for c in range(KC):
                    nc.sync.dma_start(out=wv_all[:, c, :],
                                      in_=io["wvT"][c * P:(c + 1) * P, :])

    # output-projection weights: land during the V/attention phases
    wo_t = []
    for i in range(3):
        w = const.tile([P, D], PV_DT, name=f"wo_t{i}", tag=f"wo_t{i}")
        nc.sync.dma_start(out=w[:], in_=io["woT"][i * P:(i + 1) * P, :])
        wo_t.append(w)

    # attention pools: score/exp PSUM + PV accumulators (pv banks reserved
    # through V so head-0 PV can interleave as soon as vst chunks land)
    p_pool = ctx.enter_context(tc.tile_pool(name="p_pool", bufs=16))
    aout_pool = ctx.enter_context(tc.tile_pool(name="aout_pool", bufs=2))
    aN_pool = ctx.enter_context(tc.tile_pool(name="aN_pool", bufs=1))
    ot_pool = ctx.enter_context(tc.tile_pool(name="ot_pool", bufs=3))
    attn_stack = ExitStack()
    s_ps = attn_stack.enter_context(
        tc.tile_pool(name="s_ps", bufs=2, space="PSUM"))
    pv_ps = attn_stack.enter_context(
        tc.tile_pool(name="pv_ps", bufs=2, space="PSUM"))

    PTS = {}

    def emit_schunk(h, kc):
        st = s_ps.tile([P, TQ], F32, name="st", tag="s")
        lhs = kT[h][:, kc * P:(kc + 1) * P]
        nc.tensor.matmul(st[:, 0:512], lhs, qT[h][:, 0:512])
        nc.tensor.matmul(st[:, 512:1024], lhs, qT[h][:, 512:1024])
        pt = p_pool.tile([P, TQ], PV_DT, name="pt", tag="pt")
        nc.scalar.activation(pt[:], st[:], Exp, scale=SCALE)
        PTS[(h, kc)] = pt

    PRE = 16

    # ---- phase V: v natural [Tk, 4*97] (c-outer, by Tk m-pair) -----------
    with ExitStack() as vctx:
        psv_pool = vctx.enter_context(
            tc.tile_pool(name="psv", bufs=2, space="PSUM"))
        mv_pool = vctx.enter_context(tc.tile_pool(name="mhv", bufs=16))
        mem_v = io["memTv"] if SPLIT_MEM else io["memT"]
        for q8 in range(8):
            col = q8 * 256
            psv = [psv_pool.tile([P, HG], F32, name=f"psv{q8}_{i}", tag="psv")
                   for i in range(2)]
            for c in range(KC):
                mct = mv_pool.tile([P, 256], PV_DT, name="mcv", tag="mcv")
                nc.sync.dma_start(
                    out=mct[:],
                    in_=mem_v[c * P:(c + 1) * P, col:col + 256])
                for ml in range(2):
                    nc.tensor.matmul(
                        psv[ml][:], mct[:, ml * P:(ml + 1) * P], wv_all[:, c, :],
                        start=(c == 0), stop=(c == KC - 1))
            for ml in range(2):
                mg = q8 * 2 + ml
                dst = vst[mg].rearrange("p (h c) -> p h c", c=VW)[:, :, 0:HD]
                src = psv[ml].rearrange("p (h c) -> p h c", c=HD)
                nc.vector.tensor_copy(dst, src)
            if 1 <= q8 <= 7:     # pre-compute head 0 scores under V
                lo = [0, 0, 2, 4, 7, 10, 12, 14][q8]
                hi = [0, 2, 4, 7, 10, 12, 14, 16][q8]
                for k in range(lo, hi):
                    emit_schunk(0, k)

    # ones columns of v
    for m in range(NKC):
        ones_cols = vst[m].rearrange("p (h c) -> p h c", c=VW)[:, :, HD:HD + 1]
        nc.sync.dma_start(out=ones_cols,
                          in_=io["ones4"][:].rearrange("p (h c) -> p h c", c=1))

    # denominator-broadcast PSUM pool (opens after V frees its banks)
    den_ps = attn_stack.enter_context(
        tc.tile_pool(name="den_ps", bufs=2, space="PSUM"))

    # ---- attention: one flat pipeline across all (head, chunk) pairs -----
    # PV lags S/exp by 3 chunks and flows straight across head boundaries,
    # so the ACT engine never drains between heads.

    # aoutN stacked as 3 tiles of 128 partitions (heads packed) so the
    # o-projection contracts in 3 chunks of 128 instead of 4 of 96
    aN = [aN_pool.tile([P, TQ], PV_DT, name=f"aN{i}", tag=f"aN{i}")
          for i in range(3)]
    # per-head write segments: (tile, tile_row0, head_row0, nrows)
    _SEG = {0: [(0, 0, 0, 96)],
            1: [(0, 96, 0, 32), (1, 0, 32, 32), (1, 32, 64, 32)],
            2: [(1, 64, 0, 64), (2, 0, 64, 32)],
            3: [(2, 32, 0, 32), (2, 64, 32, 32), (2, 96, 64, 32)]}

    pvs = {}

    def finish_head(h):
        pv0, pv1 = pvs.pop(h)
        den1 = tmp_pool.tile([1, TQ], F32, name="den1", tag="den1")
        nc.vector.tensor_copy(den1[:, 0:512], pv0[HD:HD + 1, :])
        nc.vector.tensor_copy(den1[:, 512:1024], pv1[HD:HD + 1, :])
        rec1f = tmp_pool.tile([1, TQ], F32, name="rec1f", tag="rec1f")
        nc.vector.reciprocal_approx_fast(out=rec1f[:], in_=den1[:])
        rec1 = tmp_pool.tile([1, TQ], PV_DT, name="rec1", tag="rec1")
        nc.vector.tensor_copy(rec1[:], rec1f[:])
        aout = aout_pool.tile([VW, TQ], PV_DT, name="aout", tag="aout")
        nc.scalar.copy(aout[:, 0:512], pv0[:])
        nc.vector.tensor_copy(aout[:, 512:1024], pv1[:])
        denB = [den_ps.tile([P, 512], F32, name=f"denB{h}_{n}", tag="denB")
                for n in range(2)]
        for n in range(2):
            nc.tensor.matmul(denB[n][:], ones1_t[:],
                             rec1[:, n * 512:(n + 1) * 512])
        for (ti, tr, hr, nr) in _SEG[h]:
            for n in range(2):
                nc.vector.tensor_mul(
                    aN[ti][tr:tr + nr, n * 512:(n + 1) * 512],
                    aout[hr:hr + nr, n * 512:(n + 1) * 512],
                    denB[n][hr:hr + nr, :])

    LAG = 3
    G = NH_CORE * NKC
    for g in range(G + LAG):
        if g < G:
            h, kc = divmod(g, NKC)
            if not (h == 0 and kc < PRE):
                emit_schunk(h, kc)
        if g >= LAG:
            h2, kc2 = divmod(g - LAG, NKC)
            if kc2 == 0:
                pvs[h2] = (
                    pv_ps.tile([VW, 512], F32, name=f"pv{h2}0", tag="pv"),
                    pv_ps.tile([VW, 512], F32, name=f"pv{h2}1", tag="pv"))
            pv0, pv1 = pvs[h2]
            pt = PTS[(h2, kc2)]
            vl = vst[kc2][:, h2 * VW:(h2 + 1) * VW]
            first, last = (kc2 == 0), (kc2 == NKC - 1)
            nc.tensor.matmul(pv0[:], vl, pt[:, 0:512], start=first, stop=last)
            nc.tensor.matmul(pv1[:], vl, pt[:, 512:1024],
                             start=first, stop=last)
            PTS.pop((h2, kc2))
            if last:
                finish_head(h2)

    attn_stack.close()

    # ---- output projection (double-buffered m-tiles, 4 PSUM banks) -------
    with ExitStack() as octx:
        o_ps = octx.enter_context(
            tc.tile_pool(name="o_ps", bufs=4, space="PSUM"))
        for m in range(MTILES):
            po0 = o_ps.tile([P, 512], F32, name=f"po{m}_0", tag="po")
            po1 = o_ps.tile([P, 512], F32, name=f"po{m}_1", tag="po")
            for i in range(3):
                lhs = wo_t[i][:, m * P:(m + 1) * P]
                nc.tensor.matmul(po0[:], lhs, aN[i][:, 0:512],
                                 start=(i == 0), stop=(i == 2))
                nc.tensor.matmul(po1[:], lhs, aN[i][:, 512:1024],
                                 start=(i == 0), stop=(i == 2))
            ot = ot_pool.tile([P, TQ], F32, name="ot", tag="ot")
            nc.vector.tensor_copy(ot[:, 0:512], po0[:])
            nc.sync.dma_start(out=oT[m * P:(m + 1) * P, 0:512],
                              in_=ot[:, 0:512])
            nc.scalar.copy(ot[:, 512:1024], po1[:])
            nc.sync.dma_start(out=oT[m * P:(m + 1) * P, 512:1024],
                              in_=ot[:, 512:1024])


# ---------------------------------------------------------------- host side
def _rope_tables(coords, T):
    """Feature-major cos/sin tables [HD, T] with the sign fold.

    Row j < 48 of the rotated output is q[j]*cos_j - q[j+48]*sin_j and row
    j >= 48 is q[j]*cos_{j-48} + q[j-48]*sin_{j-48}; the device computes
    rot = q * cE + swap(q) * sE with swap(q)[j] = q[(j+48) % 96].
    """
    coords = np.asarray(coords, np.float32)
    inv_freq = (1.0 / (ROPE_BASE ** (np.arange(FREQ_PER_AXIS, dtype=np.float32)
                                     / FREQ_PER_AXIS))).astype(np.float32)
    ang = coords[:, :, None] * inv_freq[None, None, :]   # [T, 3, 16]
    ang = ang.reshape(T, ROPE_HALF)                      # [T, 48]
    sin = np.sin(ang).astype(np.float32).T               # [48, T]
    cos = np.cos(ang).astype(np.float32).T
    cE = np.concatenate([cos, cos], axis=0)              # [96, T]
    sE = np.concatenate([-sin, sin], axis=0)
    return np.ascontiguousarray(cE), np.ascontiguousarray(sE)


def _make_in_maps(inputs):
    x = np.asarray(inputs["x"], np.float32)
    memory = np.asarray(inputs["memory"], np.float32)
    qc = np.asarray(inputs["query_coords"], np.float32)
    mc = np.asarray(inputs["memory_coords"], np.float32)
    Wq = np.asarray(inputs["Wq"], np.float32)
    Wk = np.asarray(inputs["Wk"], np.float32)
    Wv = np.asarray(inputs["Wv"], np.float32)
    Wo = np.asarray(inputs["Wo"], np.float32)
    bq = np.asarray(inputs["bq"], np.float32)
    bk = np.asarray(inputs["bk"], np.float32)

    WqT = np.ascontiguousarray(Wq.T).astype(QK_NP)   # [in, out]
    WkT = np.ascontiguousarray(Wk.T).astype(QK_NP)
    WvT = np.ascontiguousarray(Wv.T).astype(PV_NP)
    WoT = np.ascontiguousarray(Wo.T).astype(PV_NP)

    per_batch = []
    for b in range(B):
        cqE, sqE = _rope_tables(qc[b], TQ)
        ckE, skE = _rope_tables(mc[b], TK)
        entry = {
            "xT": np.ascontiguousarray(x[b].T).astype(QK_NP),
            "memT": np.ascontiguousarray(memory[b].T).astype(QK_NP),
            "cqE": cqE, "sqE": sqE, "ckE": ckE, "skE": skE,
        }
        if SPLIT_MEM:
            entry["memTv"] = np.ascontiguousarray(memory[b].T).astype(PV_NP)
        per_batch.append(entry)

    in_maps = []
    for core in range(N_CORES):
        b, g = divmod(core, NH_CORE)
        sl = slice(g * HG, (g + 1) * HG)
        m = dict(per_batch[b])
        m["wqT"] = np.ascontiguousarray(WqT[:, sl])
        m["wkT"] = np.ascontiguousarray(WkT[:, sl])
        m["wvT"] = np.ascontiguousarray(WvT[:, sl])
        m["woT"] = np.ascontiguousarray(WoT[sl, :])
        m["bq4"] = np.ascontiguousarray(bq[sl].reshape(NH_CORE, HD).T)
        m["bk4"] = np.ascontiguousarray(bk[sl].reshape(NH_CORE, HD).T)
        m["ones1"] = np.ones((1, 128), PV_NP)
        m["ones4"] = np.ones((128, NH_CORE), PV_NP)
        in_maps.append(m)
    return in_maps


def _assemble(results, inputs):
    Wo = np.asarray(inputs["Wo"], np.float32)
    bv = np.asarray(inputs["bv"], np.float32)
    bo = np.asarray(inputs["bo"], np.float32)
    cvec = (bv @ Wo.T + bo).astype(np.float32)   # exact: attn rows sum to 1
    out = np.empty((B, TQ, D), np.float32)
    for b in range(B):
        acc = np.zeros((D, TQ), np.float64)
        for g in range(NH_CORE):
            acc += results[b * NH_CORE + g]["oT"]
        out[b] = acc.T.astype(np.float32) + cvec
    return out


_NC_CACHE = None


def _get_nc():
    global _NC_CACHE
    if _NC_CACHE is None:
        _NC_CACHE = _build_nc()
    return _NC_CACHE


_RUNNER = None


def _get_runner():
    """Reusable jitted PJRT executable (same lowering run_bass_kernel_spmd
    uses under axon) so repeated kernel() calls skip recompilation."""
    global _RUNNER
    if _RUNNER is not None:
        return _RUNNER
    import jax
    from jax.sharding import Mesh, PartitionSpec
    try:
        from jax.experimental.shard_map import shard_map
    except ImportError:
        from jax import shard_map
    from concourse import bass2jax

    nc = _get_nc()
    bass2jax.install_neuronx_cc_hook()
    partition_name = (nc.partition_id_tensor.name
                      if nc.partition_id_tensor else None)
    in_names, out_names, out_avals, zero_outs = [], [], [], []
    for alloc in nc.m.functions[0].allocations:
        if not isinstance(alloc, mybir.MemoryLocationSet):
            continue
        name = alloc.memorylocations[0].name
        if alloc.kind == "ExternalInput":
            if name != partition_name:
                in_names.append(name)
        elif alloc.kind == "ExternalOutput":
            out_names.append(name)
            shape = tuple(alloc.tensor_shape)
            dtype = mybir.dt.np(alloc.dtype)
            out_avals.append(jax.core.ShapedArray(shape, dtype))
            zero_outs.append(np.zeros(shape, dtype))
    n_params = len(in_names)
    all_in = list(in_names) + list(out_names)
    if partition_name is not None:
        all_in.append(partition_name)

    def _b(*args):
        operands = list(args)
        if partition_name is not None:
            operands.append(bass2jax.partition_id_tensor())
        return tuple(bass2jax._bass_exec_p.bind(
            *operands, out_avals=tuple(out_avals), in_names=tuple(all_in),
            out_names=tuple(out_names), lowering_input_output_aliases=(),
            sim_require_finite=True, sim_require_nnan=True, nc=nc))

    devices = jax.devices()[:N_CORES]
    mesh = Mesh(np.asarray(devices), ("core",))
    nio = n_params + len(out_avals)
    fn = jax.jit(shard_map(_b, mesh=mesh,
                           in_specs=(PartitionSpec("core"),) * nio,
                           out_specs=(PartitionSpec("core"),) * len(out_avals),
                           check_rep=False), keep_unused=True)

    def run(in_maps):
        per_core = [[np.asarray(m[n]) for n in in_names] for m in in_maps]
        concat_in = [np.concatenate([per_core[c][i] for c in range(N_CORES)],
                                    axis=0) for i in range(n_params)]
        concat_zeros = [np.zeros((N_CORES * z.shape[0], *z.shape[1:]), z.dtype)
                        for z in zero_outs]
        outs = fn(*concat_in, *concat_zeros)
        return [
            {name: np.asarray(outs[i]).reshape(N_CORES, *out_avals[i].shape)[c]
             for i, name in enumerate(out_names)}
            for c in range(N_CORES)
        ]

    _RUNNER = run
    return run


_CALLED = False


def kernel(**inputs) -> np.ndarray:
    """Full-input entry point: shards across 8 NeuronCores, runs the Bass
    kernel, gathers and unshards. First call uses run_bass_kernel_spmd
    (compile + run); later calls reuse the cached executable."""
    global _CALLED
    in_maps = _make_in_maps(inputs)
    if not _CALLED:
        _CALLED = True
        nc = _get_nc()
        res = run_bass_kernel_spmd(nc, in_maps, list(range(N_CORES)))
        results = res.results
    else:
        results = _get_runner()(in_maps)
    return _assemble(results, inputs)


# revision 10
# speedup vs baseline: 1.1520x; 1.1520x over previous
"""Trainium2 Bass kernel for a cross-attention block with 3D-coordinate RoPE.

Module: q/k/v projections of x [B,Tq,D] against memory [B,Tk,D], 3D-coord
rotary embedding on q/k, softmax(q k^T / sqrt(Hd)) v, output projection.
B=2, Tq=1024, Tk=2048, D=1536, 16 heads x 96.

Sharding: 8 cores = (2 batches) x (4 head-groups of 4 heads). Each core
computes its heads end-to-end plus a partial output projection; the host
sums the 4 partials per batch. Biases bv/bo are folded in on the host
(attention rows sum to one), bq/bk are added on-device during PSUM
eviction.

Layout: feature-major ("transposed") on device. Scores are computed
transposed (S^T = k q^T) so the PV matmul needs no on-chip transposes;
softmax denominators come from a ones-column appended to v; the
per-query normalization is broadcast across partitions with a K=1
ones-vector matmul of the reciprocal row.

Schedule notes (v3):
- memory^T is resident in SBUF (full 4KB-line row loads), but its DMAs
  are emitted after Q's critical x/Wq loads so the Q phase stays
  compute-bound; the K phase paces per-chunk with the arrivals.
- the RoPE half-rotation swap is two SBUF->SBUF DMA copies per tile
  (partition remap) instead of a PE permutation matmul; the combine is
  three in-place DVE ops. TensorE runs zero RoPE columns. All dependent
  (swap) DMAs are emitted after every independent HBM load: HWDGE
  queues are FIFO, so a descriptor waiting on a semaphore blocks its
  whole queue.
- per-head softmax normalization takes the reciprocal of the [1,Tq]
  denominator row first, then broadcasts it with the ones-matmul, so the
  post-PV critical path is two tiny DVE ops + one 1024-col matmul.
- O-projection accumulates in a dedicated 4-bank PSUM pool (two m-tiles
  in flight).

Matmul dtype is selectable via KMM_DTYPE in {bf16, f16, f32r, f32};
logits, softmax and denominators stay fp32 in all modes.
"""

import os
import sys

sys.path.insert(0, "/opt/trn_rl_repo")

import numpy as np
import ml_dtypes
from contextlib import ExitStack

import concourse.bass as bass
import concourse.tile as tile
from concourse import bacc, mybir
from concourse.bass_utils import run_bass_kernel_spmd

# ---------------------------------------------------------------- constants
B = 2
TQ = 1024
TK = 2048
D = 1536
NH = 16
HD = 96
ROPE_HALF = HD // 2           # 48
FREQ_PER_AXIS = ROPE_HALF // 3  # 16
ROPE_BASE = 10000.0
NH_CORE = 4                   # heads per core
HG = NH_CORE * HD             # 384 features per core
KC = D // 128                 # 12 contraction chunks
MTILES = D // 128             # 12 output-row tiles of the o-projection
SCALE = 1.0 / float(np.sqrt(HD))
N_CORES = 8
VW = HD + 1                   # 97: head-dim + ones column

F32 = mybir.dt.float32

_MM_DT_NAME = os.environ.get("KMM_DTYPE", "f16")
_DT = {"f32r": mybir.dt.float32r, "f32": mybir.dt.float32,
       "bf16": mybir.dt.bfloat16, "f16": mybir.dt.float16}
_NP = {"f32r": np.float32, "f32": np.float32, "bf16": ml_dtypes.bfloat16,
       "f16": np.float16}
if _MM_DT_NAME == "mixed":          # q/k chain fp32r, v/attn-weight/out bf16
    _QK_NAME, _PV_NAME = "f32r", "bf16"
elif _MM_DT_NAME == "mixed16":      # q/k chain fp32r, rest fp16
    _QK_NAME, _PV_NAME = "f32r", "f16"
else:
    _QK_NAME = _PV_NAME = _MM_DT_NAME
QK_DT, QK_NP = _DT[_QK_NAME], _NP[_QK_NAME]
PV_DT, PV_NP = _DT[_PV_NAME], _NP[_PV_NAME]
SPLIT_MEM = _QK_NAME != _PV_NAME    # ship memory twice (per-dtype) if mixed


# ---------------------------------------------------------------- bass build
def _build_nc():
    nc = bacc.Bacc(trn_type="TRN2", target_bir_lowering=False, debug=False)

    io = {}
    def dram_in(name, shape, dt):
        io[name] = nc.dram_tensor(name, list(shape), dt, kind="ExternalInput").ap()
    dram_in("xT", [D, TQ], QK_DT)
    dram_in("memT", [D, TK], QK_DT)
    if SPLIT_MEM:
        dram_in("memTv", [D, TK], PV_DT)
    dram_in("wqT", [D, HG], QK_DT)  # columns of Wq^T for this head group
    dram_in("wkT", [D, HG], QK_DT)
    dram_in("wvT", [D, HG], PV_DT)
    dram_in("woT", [HG, D], PV_DT)  # rows of Wo^T for this head group
    dram_in("bq4", [HD, NH_CORE], F32)
    dram_in("bk4", [HD, NH_CORE], F32)
    dram_in("cqE", [HD, TQ], F32)  # cos table, feature-major, q side
    dram_in("sqE", [HD, TQ], F32)  # sign-folded sin table, q side
    dram_in("ckE", [HD, TK], F32)
    dram_in("skE", [HD, TK], F32)
    dram_in("ones1", [1, 128], PV_DT)
    dram_in("ones4", [128, NH_CORE], PV_DT)
    oT = nc.dram_tensor("oT", [D, TQ], F32, kind="ExternalOutput").ap()

    with tile.TileContext(nc) as tc, ExitStack() as ctx:
        _body(ctx, tc, io, oT)
    nc.compile()
    return nc


def _body(ctx, tc, io, oT):
    nc = tc.nc
    P = 128
    NKC = TK // P
    Exp = mybir.ActivationFunctionType.Exp
    Ident = mybir.ActivationFunctionType.Identity

    const = ctx.enter_context(tc.tile_pool(name="const", bufs=1))
    resident = ctx.enter_context(tc.tile_pool(name="resident", bufs=1))

    ones1_t = const.tile([1, P], PV_DT, name="ones1_t")
    bq_t = const.tile([HD, NH_CORE], F32, name="bq_t")
    bk_t = const.tile([HD, NH_CORE], F32, name="bk_t")
    cq_t = const.tile([HD, TQ], F32, name="cq_t")
    sq_t = const.tile([HD, TQ], F32, name="sq_t")
    ck_t = const.tile([HD, TK], F32, name="ck_t")
    sk_t = const.tile([HD, TK], F32, name="sk_t")
    wk_all = const.tile([P, KC, HG], QK_DT, name="wk_all")
    wv_all = const.tile([P, KC, HG], PV_DT, name="wv_all")
    wo_t = [const.tile([P, D], PV_DT, name=f"wo_t{i}", tag=f"wo_t{i}")
            for i in range(3)]
    qT = [resident.tile([HD, TQ], QK_DT, name=f"qT{h}", tag=f"qT{h}")
          for h in range(NH_CORE)]
    kT = [resident.tile([HD, TK], QK_DT, name=f"kT{h}", tag=f"kT{h}")
          for h in range(NH_CORE)]
    qsw = [resident.tile([HD, TQ], QK_DT, name=f"qsw{h}", tag=f"qsw{h}")
           for h in range(NH_CORE)]
    ksw = [resident.tile([HD, TK // 2], QK_DT, name=f"ksw{h}", tag=f"ksw{h}")
           for h in range(NH_CORE)]   # half-width, reused across K halves
    vst = [resident.tile([P, NH_CORE * VW], PV_DT, name=f"vst{m}", tag=f"vst{m}")
           for m in range(NKC)]
    tmp_pool = ctx.enter_context(tc.tile_pool(name="tmp_pool", bufs=1))
    p_pool = ctx.enter_context(tc.tile_pool(name="p_pool", bufs=14))
    # full memory^T resident (2-byte dtypes only): loaded once, serves K and V
    MEM_RES = mybir.dt.size(QK_DT) == 2 and not SPLIT_MEM
    mem_stack = ExitStack()
    memR = None
    if MEM_RES:
        mem_pool = mem_stack.enter_context(tc.tile_pool(name="mem_pool", bufs=1))
        memR = [mem_pool.tile([P, TK], QK_DT, name=f"memR{c}", tag=f"memR{c}")
                for c in range(KC)]

    # ---- phase Q: q^T = Wq_h @ x^T (c-outer, 8 psum banks) ---------------
    with ExitStack() as qctx:
        psq_pool = qctx.enter_context(
            tc.tile_pool(name="psq", bufs=NH_CORE, space="PSUM"))
        xq_pool = qctx.enter_context(tc.tile_pool(name="xq", bufs=4))
        wq_pool = qctx.enter_context(tc.tile_pool(name="wq", bufs=1))
        wq_all = wq_pool.tile([P, KC, HG], QK_DT, name="wq_all")
        psq = [psq_pool.tile([HD, TQ], F32, name=f"psq{h}", tag="psq")
               for h in range(NH_CORE)]
        for c in range(KC):
            nc.sync.dma_start(out=wq_all[:, c, :],
                              in_=io["wqT"][c * P:(c + 1) * P, :])
            xc = xq_pool.tile([P, TQ], QK_DT, name="xc", tag="xc")
            nc.sync.dma_start(out=xc[:, 0:512],
                              in_=io["xT"][c * P:(c + 1) * P, 0:512])
            nc.sync.dma_start(out=xc[:, 512:1024],
                              in_=io["xT"][c * P:(c + 1) * P, 512:1024])

            for h in range(NH_CORE):
                lhs = wq_all[:, c, h * HD:(h + 1) * HD]
                for n in range(2):
                    nc.tensor.matmul(
                        psq[h][:, n * 512:(n + 1) * 512],
                        lhs, xc[:, n * 512:(n + 1) * 512],
                        start=(c == 0), stop=(c == KC - 1))
        nc.sync.dma_start(out=bq_t[:], in_=io["bq4"][:])
        # independent loads for K and beyond, in consumption order (all
        # emitted before any dependent SBUF->SBUF DMA: HWDGE FIFO)
        for c in range(KC):
            nc.sync.dma_start(out=wk_all[:, c, :],
                              in_=io["wkT"][c * P:(c + 1) * P, :])
            if MEM_RES:
                nc.sync.dma_start(out=memR[c][:],
                                  in_=io["memT"][c * P:(c + 1) * P, :])
        nc.sync.dma_start(out=cq_t[:], in_=io["cqE"][:])
        nc.sync.dma_start(out=sq_t[:], in_=io["sqE"][:])
        nc.sync.dma_start(out=bk_t[:], in_=io["bk4"][:])
        nc.sync.dma_start(out=ck_t[:], in_=io["ckE"][:])
        nc.sync.dma_start(out=sk_t[:], in_=io["skE"][:])
        nc.sync.dma_start(out=ones1_t[:], in_=io["ones1"][:])
        for c in range(KC):
            nc.sync.dma_start(out=wv_all[:, c, :],
                              in_=io["wvT"][c * P:(c + 1) * P, :])
        for i in range(3):
            nc.sync.dma_start(out=wo_t[i][:],
                              in_=io["woT"][i * P:(i + 1) * P, :])
        for m in range(NKC):
            ones_cols = (vst[m].rearrange("p (h c) -> p h c", c=VW)
                         [:, :, HD:HD + 1])
            nc.sync.dma_start(
                out=ones_cols,
                in_=io["ones4"][:].rearrange("p (h c) -> p h c", c=1))
        for h in range(NH_CORE):
            if h % 2 == 0:
                nc.vector.tensor_scalar_add(qT[h][:], psq[h][:],
                                            bq_t[:, h:h + 1])
            else:
                nc.scalar.activation(qT[h][:], psq[h][:], Ident,
                                     bias=bq_t[:, h:h + 1])

    def rope_dma(dst, sw, cE, sE, lo, width, sw_lo=None):
        """Rotate dst[:, lo:lo+width] in place: swap halves via SBUF->SBUF
        DMA into sw, then dst = dst*cE + sw*sE (three in-place DVE ops)."""
        sl = slice(lo, lo + width)
        if sw_lo is None:
            sw_lo = lo
        sw_sl = slice(sw_lo, sw_lo + width)
        nc.sync.dma_start(out=sw[0:ROPE_HALF, sw_sl],
                          in_=dst[ROPE_HALF:HD, sl])
        nc.sync.dma_start(out=sw[ROPE_HALF:HD, sw_sl],
                          in_=dst[0:ROPE_HALF, sl])
        nc.vector.tensor_mul(dst[:, sl], dst[:, sl], cE[:, sl])
        nc.vector.tensor_mul(sw[:, sw_sl], sw[:, sw_sl], sE[:, sl])
        nc.vector.tensor_add(dst[:, sl], dst[:, sl], sw[:, sw_sl])

    # rope q: runs on DMA+DVE entirely, overlapping the K matmuls
    for h in range(NH_CORE):
        rope_dma(qT[h], qsw[h], cq_t, sq_t, 0, TQ)

    # ---- phase K: k^T = Wk_h @ mem^T (c-outer, by Tk quarter) ------------
    with ExitStack() as kctx:
        psk_pool = kctx.enter_context(
            tc.tile_pool(name="psk", bufs=8, space="PSUM"))
        mh_pool = kctx.enter_context(tc.tile_pool(name="mhk", bufs=4))
        for q4 in range(4):
            col = q4 * 512
            psk = [psk_pool.tile([HD, 512], F32, name=f"psk{q4}_{h}", tag="psk")
                   for h in range(NH_CORE)]
            for c in range(KC):
                if MEM_RES:
                    mc = memR[c][:, col:col + 512]
                else:
                    mct = mh_pool.tile([P, 512], QK_DT, name="mck", tag="mck")
                    nc.sync.dma_start(
                        out=mct[:],
                        in_=io["memT"][c * P:(c + 1) * P, col:col + 512])
                    mc = mct[:]
                for h in range(NH_CORE):
                    nc.tensor.matmul(
                        psk[h][:], wk_all[:, c, h * HD:(h + 1) * HD], mc,
                        start=(c == 0), stop=(c == KC - 1))
            for h in range(NH_CORE):
                if h % 2 == 0:
                    nc.vector.tensor_scalar_add(kT[h][:, col:col + 512],
                                                psk[h][:], bk_t[:, h:h + 1])
                else:
                    nc.scalar.activation(kT[h][:, col:col + 512],
                                         psk[h][:], Ident,
                                         bias=bk_t[:, h:h + 1])
            for h in range(NH_CORE):
                rope_dma(kT[h], ksw[h], ck_t, sk_t, col, 512,
                         sw_lo=col % 1024)

    # attention pools: score/exp PSUM + PV accumulators
    attn_stack = ExitStack()
    s_ps = attn_stack.enter_context(
        tc.tile_pool(name="s_ps", bufs=2, space="PSUM"))
    pv_ps = attn_stack.enter_context(
        tc.tile_pool(name="pv_ps", bufs=2, space="PSUM"))

    PTS = {}

    def emit_schunk(h, kc):
        st = s_ps.tile([P, TQ], F32, name="st", tag="s")
        lhs = kT[h][:, kc * P:(kc + 1) * P]
        nc.tensor.matmul(st[:, 0:512], lhs, qT[h][:, 0:512])
        nc.tensor.matmul(st[:, 512:1024], lhs, qT[h][:, 512:1024])
        pt = p_pool.tile([P, TQ], PV_DT, name="pt", tag="pt")
        nc.scalar.activation(pt[:], st[:], Exp, scale=SCALE)
        PTS[(h, kc)] = pt

    PRE = 16

    # ---- phase V: v natural [Tk, 4*97] (c-outer, by Tk m-pair) -----------
    with ExitStack() as vctx:
        psv_pool = vctx.enter_context(
            tc.tile_pool(name="psv", bufs=2, space="PSUM"))
        mv_pool = vctx.enter_context(tc.tile_pool(name="mhv", bufs=4))
        mem_v = io["memTv"] if SPLIT_MEM else io["memT"]
        for q8 in range(8):
            col = q8 * 256
            psv = [psv_pool.tile([P, HG], F32, name=f"psv{q8}_{i}", tag="psv")
                   for i in range(2)]
            for c in range(KC):
                if MEM_RES:
                    mc = memR[c][:, col:col + 256]
                else:
                    mct = mv_pool.tile([P, 256], PV_DT, name="mcv", tag="mcv")
                    nc.sync.dma_start(
                        out=mct[:],
                        in_=mem_v[c * P:(c + 1) * P, col:col + 256])
                    mc = mct[:]
                for ml in range(2):
                    nc.tensor.matmul(
                        psv[ml][:], mc[:, ml * P:(ml + 1) * P], wv_all[:, c, :],
                        start=(c == 0), stop=(c == KC - 1))
            for ml in range(2):
                mg = q8 * 2 + ml
                dst = vst[mg].rearrange("p (h c) -> p h c", c=VW)[:, :, 0:HD]
                src = psv[ml].rearrange("p (h c) -> p h c", c=HD)
                nc.vector.tensor_copy(dst, src)
            if 1 <= q8 <= 7:     # pre-compute head 0 scores under V
                lo = [0, 0, 2, 4, 7, 10, 12, 14][q8]
                hi = [0, 2, 4, 7, 10, 12, 14, 16][q8]
                for k in range(lo, hi):
                    emit_schunk(0, k)
    mem_stack.close()

    # denominator-broadcast PSUM pool (opens after V frees its banks)
    den_ps = attn_stack.enter_context(
        tc.tile_pool(name="den_ps", bufs=2, space="PSUM"))
    aout_pool = ctx.enter_context(tc.tile_pool(name="aout_pool", bufs=2))
    aN_pool = ctx.enter_context(tc.tile_pool(name="aN_pool", bufs=1))
    ot_pool = ctx.enter_context(tc.tile_pool(name="ot_pool", bufs=3))

    # ---- attention: one flat pipeline across all (head, chunk) pairs -----
    # PV lags S/exp by 3 chunks and flows straight across head boundaries,
    # so the ACT engine never drains between heads.

    # aoutN stacked as 3 tiles of 128 partitions (heads packed) so the
    # o-projection contracts in 3 chunks of 128 instead of 4 of 96
    aN = [aN_pool.tile([P, TQ], PV_DT, name=f"aN{i}", tag=f"aN{i}")
          for i in range(3)]
    # per-head write segments: (tile, tile_row0, head_row0, nrows)
    _SEG = {0: [(0, 0, 0, 96)],
            1: [(0, 96, 0, 32), (1, 0, 32, 32), (1, 32, 64, 32)],
            2: [(1, 64, 0, 64), (2, 0, 64, 32)],
            3: [(2, 32, 0, 32), (2, 64, 32, 32), (2, 96, 64, 32)]}

    pvs = {}

    def finish_head(h):
        pv0, pv1 = pvs.pop(h)
        den1 = tmp_pool.tile([1, TQ], F32, name="den1", tag="den1")
        nc.vector.tensor_copy(den1[:, 0:512], pv0[HD:HD + 1, :])
        nc.vector.tensor_copy(den1[:, 512:1024], pv1[HD:HD + 1, :])
        rec1f = tmp_pool.tile([1, TQ], F32, name="rec1f", tag="rec1f")
        nc.vector.reciprocal_approx_fast(out=rec1f[:], in_=den1[:])
        rec1 = tmp_pool.tile([1, TQ], PV_DT, name="rec1", tag="rec1")
        nc.vector.tensor_copy(rec1[:], rec1f[:])
        aout = aout_pool.tile([VW, TQ], PV_DT, name="aout", tag="aout")
        nc.scalar.copy(aout[:, 0:512], pv0[:])
        nc.vector.tensor_copy(aout[:, 512:1024], pv1[:])
        denB = [den_ps.tile([P, 512], F32, name=f"denB{h}_{n}", tag="denB")
                for n in range(2)]
        for n in range(2):
            nc.tensor.matmul(denB[n][:], ones1_t[:],
                             rec1[:, n * 512:(n + 1) * 512])
        for (ti, tr, hr, nr) in _SEG[h]:
            for n in range(2):
                nc.vector.tensor_mul(
                    aN[ti][tr:tr + nr, n * 512:(n + 1) * 512],
                    aout[hr:hr + nr, n * 512:(n + 1) * 512],
                    denB[n][hr:hr + nr, :])

    LAG = 3
    G = NH_CORE * NKC
    for g in range(G + LAG):
        if g < G:
            h, kc = divmod(g, NKC)
            if not (h == 0 and kc < PRE):
                emit_schunk(h, kc)
        if g >= LAG:
            h2, kc2 = divmod(g - LAG, NKC)
            if kc2 == 0:
                pvs[h2] = (
                    pv_ps.tile([VW, 512], F32, name=f"pv{h2}0", tag="pv"),
                    pv_ps.tile([VW, 512], F32, name=f"pv{h2}1", tag="pv"))
            pv0, pv1 = pvs[h2]
            pt = PTS[(h2, kc2)]
            vl = vst[kc2][:, h2 * VW:(h2 + 1) * VW]
            first, last = (kc2 == 0), (kc2 == NKC - 1)
            nc.tensor.matmul(pv0[:], vl, pt[:, 0:512], start=first, stop=last)
            nc.tensor.matmul(pv1[:], vl, pt[:, 512:1024],
                             start=first, stop=last)
            PTS.pop((h2, kc2))
            if last:
                finish_head(h2)

    attn_stack.close()

    # ---- output projection (double-buffered m-tiles, 4 PSUM banks) -------
    with ExitStack() as octx:
        o_ps = octx.enter_context(
            tc.tile_pool(name="o_ps", bufs=4, space="PSUM"))
        for m in range(MTILES):
            po0 = o_ps.tile([P, 512], F32, name=f"po{m}_0", tag="po")
            po1 = o_ps.tile([P, 512], F32, name=f"po{m}_1", tag="po")
            for i in range(3):
                lhs = wo_t[i][:, m * P:(m + 1) * P]
                nc.tensor.matmul(po0[:], lhs, aN[i][:, 0:512],
                                 start=(i == 0), stop=(i == 2))
                nc.tensor.matmul(po1[:], lhs, aN[i][:, 512:1024],
                                 start=(i == 0), stop=(i == 2))
            ot = ot_pool.tile([P, TQ], F32, name="ot", tag="ot")
            nc.vector.tensor_copy(ot[:, 0:512], po0[:])
            nc.sync.dma_start(out=oT[m * P:(m + 1) * P, 0:512],
                              in_=ot[:, 0:512])
            nc.scalar.copy(ot[:, 512:1024], po1[:])
            nc.sync.dma_start(out=oT[m * P:(m + 1) * P, 512:1024],
                              in_=ot[:, 512:1024])


# ---------------------------------------------------------------- host side
def _rope_tables(coords, T):
    """Feature-major cos/sin tables [HD, T] with the sign fold.

    Row j < 48 of the rotated output is q[j]*cos_j - q[j+48]*sin_j and row
    j >= 48 is q[j]*cos_{j-48} + q[j-48]*sin_{j-48}; the device computes
    rot = q * cE + swap(q) * sE with swap(q)[j] = q[(j+48) % 96].
    """
    coords = np.asarray(coords, np.float32)
    inv_freq = (1.0 / (ROPE_BASE ** (np.arange(FREQ_PER_AXIS, dtype=np.float32)
                                     / FREQ_PER_AXIS))).astype(np.float32)
    ang = coords[:, :, None] * inv_freq[None, None, :]   # [T, 3, 16]
    ang = ang.reshape(T, ROPE_HALF)                      # [T, 48]
    sin = np.sin(ang).astype(np.float32).T               # [48, T]
    cos = np.cos(ang).astype(np.float32).T
    cE = np.concatenate([cos, cos], axis=0)              # [96, T]
    sE = np.concatenate([-sin, sin], axis=0)
    return np.ascontiguousarray(cE), np.ascontiguousarray(sE)


def _make_in_maps(inputs):
    x = np.asarray(inputs["x"], np.float32)
    memory = np.asarray(inputs["memory"], np.float32)
    qc = np.asarray(inputs["query_coords"], np.float32)
    mc = np.asarray(inputs["memory_coords"], np.float32)
    Wq = np.asarray(inputs["Wq"], np.float32)
    Wk = np.asarray(inputs["Wk"], np.float32)
    Wv = np.asarray(inputs["Wv"], np.float32)
    Wo = np.asarray(inputs["Wo"], np.float32)
    bq = np.asarray(inputs["bq"], np.float32)
    bk = np.asarray(inputs["bk"], np.float32)

    WqT = np.ascontiguousarray(Wq.T).astype(QK_NP)   # [in, out]
    WkT = np.ascontiguousarray(Wk.T).astype(QK_NP)
    WvT = np.ascontiguousarray(Wv.T).astype(PV_NP)
    WoT = np.ascontiguousarray(Wo.T).astype(PV_NP)

    per_batch = []
    for b in range(B):
        cqE, sqE = _rope_tables(qc[b], TQ)
        ckE, skE = _rope_tables(mc[b], TK)
        entry = {
            "xT": np.ascontiguousarray(x[b].T).astype(QK_NP),
            "memT": np.ascontiguousarray(memory[b].T).astype(QK_NP),
            "cqE": cqE, "sqE": sqE, "ckE": ckE, "skE": skE,
        }
        if SPLIT_MEM:
            entry["memTv"] = np.ascontiguousarray(memory[b].T).astype(PV_NP)
        per_batch.append(entry)

    in_maps = []
    for core in range(N_CORES):
        b, g = divmod(core, NH_CORE)
        sl = slice(g * HG, (g + 1) * HG)
        m = dict(per_batch[b])
        m["wqT"] = np.ascontiguousarray(WqT[:, sl])
        m["wkT"] = np.ascontiguousarray(WkT[:, sl])
        m["wvT"] = np.ascontiguousarray(WvT[:, sl])
        m["woT"] = np.ascontiguousarray(WoT[sl, :])
        m["bq4"] = np.ascontiguousarray(bq[sl].reshape(NH_CORE, HD).T)
        m["bk4"] = np.ascontiguousarray(bk[sl].reshape(NH_CORE, HD).T)
        m["ones1"] = np.ones((1, 128), PV_NP)
        m["ones4"] = np.ones((128, NH_CORE), PV_NP)
        in_maps.append(m)
    return in_maps


def _assemble(results, inputs):
    Wo = np.asarray(inputs["Wo"], np.float32)
    bv = np.asarray(inputs["bv"], np.float32)
    bo = np.asarray(inputs["bo"], np.float32)
    cvec = (bv @ Wo.T + bo).astype(np.float32)   # exact: attn rows sum to 1
    out = np.empty((B, TQ, D), np.float32)
    for b in range(B):
        acc = np.zeros((D, TQ), np.float64)
        for g in range(NH_CORE):
            acc += results[b * NH_CORE + g]["oT"]
        out[b] = acc.T.astype(np.float32) + cvec
    return out


_NC_CACHE = None


def _get_nc():
    global _NC_CACHE
    if _NC_CACHE is None:
        _NC_CACHE = _build_nc()
    return _NC_CACHE


_RUNNER = None


def _get_runner():
    """Reusable jitted PJRT executable (same lowering run_bass_kernel_spmd
    uses under axon) so repeated kernel() calls skip recompilation."""
    global _RUNNER
    if _RUNNER is not None:
        return _RUNNER
    import jax
    from jax.sharding import Mesh, PartitionSpec
    try:
        from jax.experimental.shard_map import shard_map
    except ImportError:
        from jax import shard_map
    from concourse import bass2jax

    nc = _get_nc()
    bass2jax.install_neuronx_cc_hook()
    partition_name = (nc.partition_id_tensor.name
                      if nc.partition_id_tensor else None)
    in_names, out_names, out_avals, zero_outs = [], [], [], []
    for alloc in nc.m.functions[0].allocations:
        if not isinstance(alloc, mybir.MemoryLocationSet):
            continue
        name = alloc.memorylocations[0].name
        if alloc.kind == "ExternalInput":
            if name != partition_name:
                in_names.append(name)
        elif alloc.kind == "ExternalOutput":
            out_names.append(name)
            shape = tuple(alloc.tensor_shape)
            dtype = mybir.dt.np(alloc.dtype)
            out_avals.append(jax.core.ShapedArray(shape, dtype))
            zero_outs.append(np.zeros(shape, dtype))
    n_params = len(in_names)
    all_in = list(in_names) + list(out_names)
    if partition_name is not None:
        all_in.append(partition_name)

    def _b(*args):
        operands = list(args)
        if partition_name is not None:
            operands.append(bass2jax.partition_id_tensor())
        return tuple(bass2jax._bass_exec_p.bind(
            *operands, out_avals=tuple(out_avals), in_names=tuple(all_in),
            out_names=tuple(out_names), lowering_input_output_aliases=(),
            sim_require_finite=True, sim_require_nnan=True, nc=nc))

    devices = jax.devices()[:N_CORES]
    mesh = Mesh(np.asarray(devices), ("core",))
    nio = n_params + len(out_avals)
    fn = jax.jit(shard_map(_b, mesh=mesh,
                           in_specs=(PartitionSpec("core"),) * nio,
                           out_specs=(PartitionSpec("core"),) * len(out_avals),
                           check_rep=False), keep_unused=True)

    def run(in_maps):
        per_core = [[np.asarray(m[n]) for n in in_names] for m in in_maps]
        concat_in = [np.concatenate([per_core[c][i] for c in range(N_CORES)],
                                    axis=0) for i in range(n_params)]
        concat_zeros = [np.zeros((N_CORES * z.shape[0], *z.shape[1:]), z.dtype)
                        for z in zero_outs]
        outs = fn(*concat_in, *concat_zeros)
        return [
            {name: np.asarray(outs[i]).reshape(N_CORES, *out_avals[i].shape)[c]
             for i, name in enumerate(out_names)}
            for c in range(N_CORES)
        ]

    _RUNNER = run
    return run


_CALLED = False


def kernel(**inputs) -> np.ndarray:
    """Full-input entry point: shards across 8 NeuronCores, runs the Bass
    kernel, gathers and unshards. First call uses run_bass_kernel_spmd
    (compile + run); later calls reuse the cached executable."""
    global _CALLED
    in_maps = _make_in_maps(inputs)
    if not _CALLED:
        _CALLED = True
        nc = _get_nc()
        res = run_bass_kernel_spmd(nc, in_maps, list(range(N_CORES)))
        results = res.results
    else:
        results = _get_runner()(in_maps)
    return _assemble(results, inputs)


# revision 14
# speedup vs baseline: 1.3260x; 1.1510x over previous
"""Trainium2 Bass kernel for a cross-attention block with 3D-coordinate RoPE.

Module: q/k/v projections of x [B,Tq,D] against memory [B,Tk,D], 3D-coord
rotary embedding on q/k, softmax(q k^T / sqrt(Hd)) v, output projection.
B=2, Tq=1024, Tk=2048, D=1536, 16 heads x 96.

Sharding: 8 cores = (2 batches) x (4 head-groups of 4 heads). Each core
computes its heads end-to-end plus a partial output projection; the host
sums the 4 partials per batch. Biases bv/bo are folded in on the host
(attention rows sum to one), bq/bk are added on-device during PSUM
eviction.

Layout: feature-major ("transposed") on device. Scores are computed
transposed (S^T = k q^T) so the PV matmul needs no on-chip transposes;
softmax denominators come from a ones-column appended to v; the
per-query normalization is broadcast across partitions with a K=1
ones-vector matmul of the reciprocal row.

Schedule notes (v3):
- memory^T is resident in SBUF (full 4KB-line row loads), but its DMAs
  are emitted after Q's critical x/Wq loads so the Q phase stays
  compute-bound; the K phase paces per-chunk with the arrivals.
- the RoPE half-rotation swap is two SBUF->SBUF DMA copies per tile
  (partition remap) instead of a PE permutation matmul; the combine is
  three in-place DVE ops. TensorE runs zero RoPE columns. All dependent
  (swap) DMAs are emitted after every independent HBM load: HWDGE
  queues are FIFO, so a descriptor waiting on a semaphore blocks its
  whole queue.
- per-head softmax normalization takes the reciprocal of the [1,Tq]
  denominator row first, then broadcasts it with the ones-matmul, so the
  post-PV critical path is two tiny DVE ops + one 1024-col matmul.
- O-projection accumulates in a dedicated 4-bank PSUM pool (two m-tiles
  in flight).

Matmul dtype is selectable via KMM_DTYPE in {bf16, f16, f32r, f32};
logits, softmax and denominators stay fp32 in all modes.
"""

import os
import sys

sys.path.insert(0, "/opt/trn_rl_repo")

import numpy as np
import ml_dtypes
from contextlib import ExitStack

import concourse.bass as bass
import concourse.tile as tile
from concourse import bacc, mybir
from concourse.bass_utils import run_bass_kernel_spmd

# ---------------------------------------------------------------- constants
B = 2
TQ = 1024
TK = 2048
D = 1536
NH = 16
HD = 96
ROPE_HALF = HD // 2           # 48
FREQ_PER_AXIS = ROPE_HALF // 3  # 16
ROPE_BASE = 10000.0
NH_CORE = 4                   # heads per core
HG = NH_CORE * HD             # 384 features per core
KC = D // 128                 # 12 contraction chunks
MTILES = D // 128             # 12 output-row tiles of the o-projection
SCALE = 1.0 / float(np.sqrt(HD))
N_CORES = 8
VW = HD + 1                   # 97: head-dim + ones column

F32 = mybir.dt.float32

_MM_DT_NAME = os.environ.get("KMM_DTYPE", "f16")
_DT = {"f32r": mybir.dt.float32r, "f32": mybir.dt.float32,
       "bf16": mybir.dt.bfloat16, "f16": mybir.dt.float16}
_NP = {"f32r": np.float32, "f32": np.float32, "bf16": ml_dtypes.bfloat16,
       "f16": np.float16}
if _MM_DT_NAME == "mixed":          # q/k chain fp32r, v/attn-weight/out bf16
    _QK_NAME, _PV_NAME = "f32r", "bf16"
elif _MM_DT_NAME == "mixed16":      # q/k chain fp32r, rest fp16
    _QK_NAME, _PV_NAME = "f32r", "f16"
else:
    _QK_NAME = _PV_NAME = _MM_DT_NAME
QK_DT, QK_NP = _DT[_QK_NAME], _NP[_QK_NAME]
PV_DT, PV_NP = _DT[_PV_NAME], _NP[_PV_NAME]
SPLIT_MEM = _QK_NAME != _PV_NAME    # ship memory twice (per-dtype) if mixed


# ---------------------------------------------------------------- bass build
def _build_nc():
    nc = bacc.Bacc(trn_type="TRN2", target_bir_lowering=False, debug=False)

    io = {}
    def dram_in(name, shape, dt):
        io[name] = nc.dram_tensor(name, list(shape), dt, kind="ExternalInput").ap()
    dram_in("xT", [D, TQ], QK_DT)
    dram_in("memT", [D, TK], QK_DT)
    if SPLIT_MEM:
        dram_in("memTv", [D, TK], PV_DT)
    dram_in("wqT", [D, HG], QK_DT)  # columns of Wq^T for this head group
    dram_in("wkT", [D, HG], QK_DT)
    dram_in("wvT", [D, HG], PV_DT)
    dram_in("woT", [HG, D], PV_DT)  # rows of Wo^T for this head group
    dram_in("bq4", [HD, NH_CORE], F32)
    dram_in("bk4", [HD, NH_CORE], F32)
    dram_in("cqE", [HD, TQ], F32)  # cos table, feature-major, q side
    dram_in("sqE", [HD, TQ], F32)  # sign-folded sin table, q side
    dram_in("ckE", [HD, TK], F32)
    dram_in("skE", [HD, TK], F32)
    dram_in("ones1", [1, 128], PV_DT)
    dram_in("ones4", [128, NH_CORE], PV_DT)
    oT = nc.dram_tensor("oT", [D, TQ], F32, kind="ExternalOutput").ap()

    with tile.TileContext(nc) as tc, ExitStack() as ctx:
        _body(ctx, tc, io, oT)
    nc.compile()
    return nc


def _body(ctx, tc, io, oT):
    nc = tc.nc
    P = 128
    NKC = TK // P
    Exp = mybir.ActivationFunctionType.Exp
    Ident = mybir.ActivationFunctionType.Identity

    const = ctx.enter_context(tc.tile_pool(name="const", bufs=1))
    resident = ctx.enter_context(tc.tile_pool(name="resident", bufs=1))

    ones1_t = const.tile([1, P], PV_DT, name="ones1_t")
    bq_t = const.tile([HD, NH_CORE], F32, name="bq_t")
    bk_t = const.tile([HD, NH_CORE], F32, name="bk_t")
    cq_t = const.tile([HD, TQ], F32, name="cq_t")
    sq_t = const.tile([HD, TQ], F32, name="sq_t")
    ck_t = const.tile([HD, TK], F32, name="ck_t")
    sk_t = const.tile([HD, TK], F32, name="sk_t")
    wk_all = const.tile([P, KC, HG], QK_DT, name="wk_all")
    wv_all = const.tile([P, KC, HG], PV_DT, name="wv_all")
    wo_t = [const.tile([P, D], PV_DT, name=f"wo_t{i}", tag=f"wo_t{i}")
            for i in range(3)]
    qT = [resident.tile([HD, TQ], QK_DT, name=f"qT{h}", tag=f"qT{h}")
          for h in range(NH_CORE)]
    kT = [resident.tile([HD, TK], QK_DT, name=f"kT{h}", tag=f"kT{h}")
          for h in range(NH_CORE)]
    qsw = [resident.tile([HD, TQ], QK_DT, name=f"qsw{h}", tag=f"qsw{h}")
           for h in range(NH_CORE)]
    ksw = [resident.tile([HD, TK // 2], QK_DT, name=f"ksw{h}", tag=f"ksw{h}")
           for h in range(NH_CORE)]   # half-width, reused across K halves
    vst = [resident.tile([P, NH_CORE * VW], PV_DT, name=f"vst{m}", tag=f"vst{m}")
           for m in range(NKC)]
    tmp_pool = ctx.enter_context(tc.tile_pool(name="tmp_pool", bufs=1))
    p_pool = ctx.enter_context(tc.tile_pool(name="p_pool", bufs=14))
    # full memory^T resident (2-byte dtypes only): loaded once, serves K and V
    MEM_RES = mybir.dt.size(QK_DT) == 2 and not SPLIT_MEM
    mem_stack = ExitStack()
    memR = None
    if MEM_RES:
        mem_pool = mem_stack.enter_context(tc.tile_pool(name="mem_pool", bufs=1))
        memR = [mem_pool.tile([P, TK], QK_DT, name=f"memR{c}", tag=f"memR{c}")
                for c in range(KC)]

    # ---- phase Q: q^T = Wq_h @ x^T (c-outer, 8 psum banks) ---------------
    with ExitStack() as qctx:
        psq_pool = qctx.enter_context(
            tc.tile_pool(name="psq", bufs=NH_CORE, space="PSUM"))
        xq_pool = qctx.enter_context(tc.tile_pool(name="xq", bufs=4))
        wq_pool = qctx.enter_context(tc.tile_pool(name="wq", bufs=1))
        wq_all = wq_pool.tile([P, KC, HG], QK_DT, name="wq_all")
        psq = [psq_pool.tile([HD, TQ], F32, name=f"psq{h}", tag="psq")
               for h in range(NH_CORE)]
        for c in range(KC):
            nc.sync.dma_start(out=wq_all[:, c, :],
                              in_=io["wqT"][c * P:(c + 1) * P, :])
            xc = xq_pool.tile([P, TQ], QK_DT, name="xc", tag="xc")
            nc.sync.dma_start(out=xc[:, 0:512],
                              in_=io["xT"][c * P:(c + 1) * P, 0:512])
            nc.sync.dma_start(out=xc[:, 512:1024],
                              in_=io["xT"][c * P:(c + 1) * P, 512:1024])

            for h in range(NH_CORE):
                lhs = wq_all[:, c, h * HD:(h + 1) * HD]
                for n in range(2):
                    nc.tensor.matmul(
                        psq[h][:, n * 512:(n + 1) * 512],
                        lhs, xc[:, n * 512:(n + 1) * 512],
                        start=(c == 0), stop=(c == KC - 1))
        nc.sync.dma_start(out=bq_t[:], in_=io["bq4"][:])
        # independent loads for K and beyond, in consumption order (all
        # emitted before any dependent SBUF->SBUF DMA: HWDGE FIFO)
        for c in range(KC):
            nc.sync.dma_start(out=wk_all[:, c, :],
                              in_=io["wkT"][c * P:(c + 1) * P, :])
            if MEM_RES:
                nc.sync.dma_start(out=memR[c][:],
                                  in_=io["memT"][c * P:(c + 1) * P, :])
        nc.sync.dma_start(out=cq_t[:], in_=io["cqE"][:])
        nc.sync.dma_start(out=sq_t[:], in_=io["sqE"][:])
        nc.sync.dma_start(out=bk_t[:], in_=io["bk4"][:])
        nc.sync.dma_start(out=ck_t[:], in_=io["ckE"][:])
        nc.sync.dma_start(out=sk_t[:], in_=io["skE"][:])
        nc.sync.dma_start(out=ones1_t[:], in_=io["ones1"][:])
        for c in range(KC):
            nc.sync.dma_start(out=wv_all[:, c, :],
                              in_=io["wvT"][c * P:(c + 1) * P, :])
        for i in range(3):
            nc.sync.dma_start(out=wo_t[i][:],
                              in_=io["woT"][i * P:(i + 1) * P, :])
        for m in range(NKC):
            ones_cols = (vst[m].rearrange("p (h c) -> p h c", c=VW)
                         [:, :, HD:HD + 1])
            nc.sync.dma_start(
                out=ones_cols,
                in_=io["ones4"][:].rearrange("p (h c) -> p h c", c=1))
        for h in range(NH_CORE):
            if h % 2 == 0:
                nc.vector.tensor_scalar_add(qT[h][:], psq[h][:],
                                            bq_t[:, h:h + 1])
            else:
                nc.scalar.activation(qT[h][:], psq[h][:], Ident,
                                     bias=bq_t[:, h:h + 1])

    def rope_dma(dst, sw, cE, sE, lo, width, sw_lo=None):
        """Rotate dst[:, lo:lo+width] in place: swap halves via SBUF->SBUF
        DMA into sw, then dst = dst*cE + sw*sE (three in-place DVE ops)."""
        sl = slice(lo, lo + width)
        if sw_lo is None:
            sw_lo = lo
        sw_sl = slice(sw_lo, sw_lo + width)
        nc.sync.dma_start(out=sw[0:ROPE_HALF, sw_sl],
                          in_=dst[ROPE_HALF:HD, sl])
        nc.sync.dma_start(out=sw[ROPE_HALF:HD, sw_sl],
                          in_=dst[0:ROPE_HALF, sl])
        nc.vector.tensor_mul(dst[:, sl], dst[:, sl], cE[:, sl])
        nc.vector.tensor_mul(sw[:, sw_sl], sw[:, sw_sl], sE[:, sl])
        nc.vector.tensor_add(dst[:, sl], dst[:, sl], sw[:, sw_sl])

    # rope q: runs on DMA+DVE entirely, overlapping the K matmuls
    for h in range(NH_CORE):
        rope_dma(qT[h], qsw[h], cq_t, sq_t, 0, TQ)

    # ---- phase K: k^T = Wk_h @ mem^T (c-outer, by Tk quarter) ------------
    with ExitStack() as kctx:
        psk_pool = kctx.enter_context(
            tc.tile_pool(name="psk", bufs=8, space="PSUM"))
        mh_pool = kctx.enter_context(tc.tile_pool(name="mhk", bufs=4))
        for q4 in range(4):
            col = q4 * 512
            psk = [psk_pool.tile([HD, 512], F32, name=f"psk{q4}_{h}", tag="psk")
                   for h in range(NH_CORE)]
            for c in range(KC):
                if MEM_RES:
                    mc = memR[c][:, col:col + 512]
                else:
                    mct = mh_pool.tile([P, 512], QK_DT, name="mck", tag="mck")
                    nc.sync.dma_start(
                        out=mct[:],
                        in_=io["memT"][c * P:(c + 1) * P, col:col + 512])
                    mc = mct[:]
                for h in range(NH_CORE):
                    nc.tensor.matmul(
                        psk[h][:], wk_all[:, c, h * HD:(h + 1) * HD], mc,
                        start=(c == 0), stop=(c == KC - 1))
            # evictions all on ACT so the DVE queue (rope + V evictions)
            # only ever contains ops with monotonically later dependencies
            for h in range(NH_CORE):
                nc.scalar.activation(kT[h][:, col:col + 512],
                                     psk[h][:], Ident,
                                     bias=bk_t[:, h:h + 1])
            # rope of the last quarter for heads 1-3 is emitted after the
            # V phase (its DVE ops would otherwise sit ahead of V's PSUM
            # evictions in the DVE FIFO while waiting on the K tail)
            ropes = range(1) if q4 == 3 else range(NH_CORE)
            for h in ropes:
                rope_dma(kT[h], ksw[h], ck_t, sk_t, col, 512,
                         sw_lo=col % 1024)

    # attention pools: score/exp PSUM + PV accumulators
    attn_stack = ExitStack()
    s_ps = attn_stack.enter_context(
        tc.tile_pool(name="s_ps", bufs=2, space="PSUM"))
    pv_ps = attn_stack.enter_context(
        tc.tile_pool(name="pv_ps", bufs=2, space="PSUM"))

    PTS = {}

    def emit_schunk(h, kc):
        st = s_ps.tile([P, TQ], F32, name="st", tag="s")
        lhs = kT[h][:, kc * P:(kc + 1) * P]
        nc.tensor.matmul(st[:, 0:512], lhs, qT[h][:, 0:512])
        nc.tensor.matmul(st[:, 512:1024], lhs, qT[h][:, 512:1024])
        pt = p_pool.tile([P, TQ], PV_DT, name="pt", tag="pt")
        nc.scalar.activation(pt[:], st[:], Exp, scale=SCALE)
        PTS[(h, kc)] = pt

    # aoutN stacked as 3 tiles of 128 partitions (heads packed) so the
    # o-projection contracts in 3 chunks of 128 instead of 4 of 96
    # (tiles bound after the V phase; placeholder lists until then)
    aN = []
    # per-head write segments: (tile, tile_row0, head_row0, nrows)
    _SEG = {0: [(0, 0, 0, 96)],
            1: [(0, 96, 0, 32), (1, 0, 32, 32), (1, 32, 64, 32)],
            2: [(1, 64, 0, 64), (2, 0, 64, 32)],
            3: [(2, 32, 0, 32), (2, 64, 32, 32), (2, 96, 64, 32)]}

    pvs = {}
    den_ps = [None]   # bound after the V phase frees PSUM banks

    def finish_head(h):
        pv0, pv1 = pvs.pop(h)
        den1 = tmp_pool.tile([1, TQ], F32, name="den1", tag="den1")
        nc.vector.tensor_copy(den1[:, 0:512], pv0[HD:HD + 1, :])
        nc.vector.tensor_copy(den1[:, 512:1024], pv1[HD:HD + 1, :])
        rec1f = tmp_pool.tile([1, TQ], F32, name="rec1f", tag="rec1f")
        nc.vector.reciprocal_approx_fast(out=rec1f[:], in_=den1[:])
        rec1 = tmp_pool.tile([1, TQ], PV_DT, name="rec1", tag="rec1")
        nc.vector.tensor_copy(rec1[:], rec1f[:])
        aout = aout_pool.tile([VW, TQ], PV_DT, name="aout", tag="aout")
        nc.vector.tensor_copy(aout[:, 0:512], pv0[:])
        nc.vector.tensor_copy(aout[:, 512:1024], pv1[:])
        denB = [den_ps[0].tile([P, 512], F32, name=f"denB{h}_{n}", tag="denB")
                for n in range(2)]
        for n in range(2):
            nc.tensor.matmul(denB[n][:], ones1_t[:],
                             rec1[:, n * 512:(n + 1) * 512])
        for (ti, tr, hr, nr) in _SEG[h]:
            for n in range(2):
                nc.vector.tensor_mul(
                    aN[ti][tr:tr + nr, n * 512:(n + 1) * 512],
                    aout[hr:hr + nr, n * 512:(n + 1) * 512],
                    denB[n][hr:hr + nr, :])

    def emit_pv(h, kc):
        if kc == 0:
            pvs[h] = (
                pv_ps.tile([VW, 512], F32, name=f"pv{h}0", tag="pv"),
                pv_ps.tile([VW, 512], F32, name=f"pv{h}1", tag="pv"))
        pv0, pv1 = pvs[h]
        pt = PTS.pop((h, kc))
        vl = vst[kc][:, h * VW:(h + 1) * VW]
        first, last = (kc == 0), (kc == NKC - 1)
        nc.tensor.matmul(pv0[:], vl, pt[:, 0:512], start=first, stop=last)
        nc.tensor.matmul(pv1[:], vl, pt[:, 512:1024], start=first, stop=last)
        if last:
            finish_head(h)

    # attention work interleaved under V: head-0 scores run ahead, head-0
    # PV lags the vst evictions, early head-1 scores keep the exp stream
    # (the attention pacer on ACT) fed across the V->attention boundary
    _S0 = {1: (0, 2), 2: (2, 4), 3: (4, 7), 4: (7, 10),
           5: (10, 13), 6: (13, 15), 7: (15, 16)}
    _PV0 = {2: (0, 1), 3: (1, 3), 4: (3, 5), 5: (5, 7), 6: (7, 10),
            7: (10, 13)}
    _S1 = {5: (0, 1), 6: (1, 3), 7: (3, 5)}
    S1_PRE = 5

    # ---- phase V: v natural [Tk, 4*97] (c-outer, by Tk m-pair) -----------
    with ExitStack() as vctx:
        psv_pool = vctx.enter_context(
            tc.tile_pool(name="psv", bufs=2, space="PSUM"))
        mv_pool = vctx.enter_context(tc.tile_pool(name="mhv", bufs=4))
        mem_v = io["memTv"] if SPLIT_MEM else io["memT"]
        for q8 in range(8):
            col = q8 * 256
            psv = [psv_pool.tile([P, HG], F32, name=f"psv{q8}_{i}", tag="psv")
                   for i in range(2)]
            for c in range(KC):
                if MEM_RES:
                    mc = memR[c][:, col:col + 256]
                else:
                    mct = mv_pool.tile([P, 256], PV_DT, name="mcv", tag="mcv")
                    nc.sync.dma_start(
                        out=mct[:],
                        in_=mem_v[c * P:(c + 1) * P, col:col + 256])
                    mc = mct[:]
                for ml in range(2):
                    nc.tensor.matmul(
                        psv[ml][:], mc[:, ml * P:(ml + 1) * P], wv_all[:, c, :],
                        start=(c == 0), stop=(c == KC - 1))
            for ml in range(2):
                mg = q8 * 2 + ml
                dst = vst[mg].rearrange("p (h c) -> p h c", c=VW)[:, :, 0:HD]
                src = psv[ml].rearrange("p (h c) -> p h c", c=HD)
                nc.vector.tensor_copy(dst, src)
            for k in range(*_S0.get(q8, (0, 0))):
                emit_schunk(0, k)
            for k in range(*_PV0.get(q8, (0, 0))):
                emit_pv(0, k)
            for k in range(*_S1.get(q8, (0, 0))):
                emit_schunk(1, k)
    mem_stack.close()

    # denominator-broadcast PSUM pool (opens after V frees its banks)
    den_ps[0] = attn_stack.enter_context(
        tc.tile_pool(name="den_ps", bufs=2, space="PSUM"))
    aout_pool = ctx.enter_context(tc.tile_pool(name="aout_pool", bufs=2))
    aN_pool = ctx.enter_context(tc.tile_pool(name="aN_pool", bufs=1))
    ot_pool = ctx.enter_context(tc.tile_pool(name="ot_pool", bufs=3))
    aN.extend(aN_pool.tile([P, TQ], PV_DT, name=f"aN{i}", tag=f"aN{i}")
              for i in range(3))

    # rope of the K tail for heads 1-3 (head 0's was done in the K phase)
    for h in range(1, NH_CORE):
        rope_dma(kT[h], ksw[h], ck_t, sk_t, 1536, 512, sw_lo=512)

    # ---- attention: flat pipeline across the remaining (head, chunk) ----
    for k in range(13, NKC):
        emit_pv(0, k)
    score_q = ([(1, k) for k in range(S1_PRE, NKC)]
               + [(h, k) for h in (2, 3) for k in range(NKC)])
    sc = 0
    for i, (h, kc) in enumerate([(h, k) for h in (1, 2, 3)
                                 for k in range(NKC)]):
        while sc <= i and sc < len(score_q):
            emit_schunk(*score_q[sc])
            sc += 1
        emit_pv(h, kc)

    attn_stack.close()

    # ---- output projection (double-buffered m-tiles, 4 PSUM banks) -------
    with ExitStack() as octx:
        o_ps = octx.enter_context(
            tc.tile_pool(name="o_ps", bufs=4, space="PSUM"))
        for m in range(MTILES):
            po0 = o_ps.tile([P, 512], F32, name=f"po{m}_0", tag="po")
            po1 = o_ps.tile([P, 512], F32, name=f"po{m}_1", tag="po")
            for i in range(3):
                lhs = wo_t[i][:, m * P:(m + 1) * P]
                nc.tensor.matmul(po0[:], lhs, aN[i][:, 0:512],
                                 start=(i == 0), stop=(i == 2))
                nc.tensor.matmul(po1[:], lhs, aN[i][:, 512:1024],
                                 start=(i == 0), stop=(i == 2))
            ot = ot_pool.tile([P, TQ], F32, name="ot", tag="ot")
            nc.vector.tensor_copy(ot[:, 0:512], po0[:])
            nc.sync.dma_start(out=oT[m * P:(m + 1) * P, 0:512],
                              in_=ot[:, 0:512])
            nc.scalar.copy(ot[:, 512:1024], po1[:])
            nc.sync.dma_start(out=oT[m * P:(m + 1) * P, 512:1024],
                              in_=ot[:, 512:1024])


# ---------------------------------------------------------------- host side
def _rope_tables(coords, T):
    """Feature-major cos/sin tables [HD, T] with the sign fold.

    Row j < 48 of the rotated output is q[j]*cos_j - q[j+48]*sin_j and row
    j >= 48 is q[j]*cos_{j-48} + q[j-48]*sin_{j-48}; the device computes
    rot = q * cE + swap(q) * sE with swap(q)[j] = q[(j+48) % 96].
    """
    coords = np.asarray(coords, np.float32)
    inv_freq = (1.0 / (ROPE_BASE ** (np.arange(FREQ_PER_AXIS, dtype=np.float32)
                                     / FREQ_PER_AXIS))).astype(np.float32)
    ang = coords[:, :, None] * inv_freq[None, None, :]   # [T, 3, 16]
    ang = ang.reshape(T, ROPE_HALF)                      # [T, 48]
    sin = np.sin(ang).astype(np.float32).T               # [48, T]
    cos = np.cos(ang).astype(np.float32).T
    cE = np.concatenate([cos, cos], axis=0)              # [96, T]
    sE = np.concatenate([-sin, sin], axis=0)
    return np.ascontiguousarray(cE), np.ascontiguousarray(sE)


def _make_in_maps(inputs):
    x = np.asarray(inputs["x"], np.float32)
    memory = np.asarray(inputs["memory"], np.float32)
    qc = np.asarray(inputs["query_coords"], np.float32)
    mc = np.asarray(inputs["memory_coords"], np.float32)
    Wq = np.asarray(inputs["Wq"], np.float32)
    Wk = np.asarray(inputs["Wk"], np.float32)
    Wv = np.asarray(inputs["Wv"], np.float32)
    Wo = np.asarray(inputs["Wo"], np.float32)
    bq = np.asarray(inputs["bq"], np.float32)
    bk = np.asarray(inputs["bk"], np.float32)

    WqT = np.ascontiguousarray(Wq.T).astype(QK_NP)   # [in, out]
    WkT = np.ascontiguousarray(Wk.T).astype(QK_NP)
    WvT = np.ascontiguousarray(Wv.T).astype(PV_NP)
    WoT = np.ascontiguousarray(Wo.T).astype(PV_NP)

    per_batch = []
    for b in range(B):
        cqE, sqE = _rope_tables(qc[b], TQ)
        ckE, skE = _rope_tables(mc[b], TK)
        entry = {
            "xT": np.ascontiguousarray(x[b].T).astype(QK_NP),
            "memT": np.ascontiguousarray(memory[b].T).astype(QK_NP),
            "cqE": cqE, "sqE": sqE, "ckE": ckE, "skE": skE,
        }
        if SPLIT_MEM:
            entry["memTv"] = np.ascontiguousarray(memory[b].T).astype(PV_NP)
        per_batch.append(entry)

    in_maps = []
    for core in range(N_CORES):
        b, g = divmod(core, NH_CORE)
        sl = slice(g * HG, (g + 1) * HG)
        m = dict(per_batch[b])
        m["wqT"] = np.ascontiguousarray(WqT[:, sl])
        m["wkT"] = np.ascontiguousarray(WkT[:, sl])
        m["wvT"] = np.ascontiguousarray(WvT[:, sl])
        m["woT"] = np.ascontiguousarray(WoT[sl, :])
        m["bq4"] = np.ascontiguousarray(bq[sl].reshape(NH_CORE, HD).T)
        m["bk4"] = np.ascontiguousarray(bk[sl].reshape(NH_CORE, HD).T)
        m["ones1"] = np.ones((1, 128), PV_NP)
        m["ones4"] = np.ones((128, NH_CORE), PV_NP)
        in_maps.append(m)
    return in_maps


def _assemble(results, inputs):
    Wo = np.asarray(inputs["Wo"], np.float32)
    bv = np.asarray(inputs["bv"], np.float32)
    bo = np.asarray(inputs["bo"], np.float32)
    cvec = (bv @ Wo.T + bo).astype(np.float32)   # exact: attn rows sum to 1
    out = np.empty((B, TQ, D), np.float32)
    for b in range(B):
        acc = np.zeros((D, TQ), np.float64)
        for g in range(NH_CORE):
            acc += results[b * NH_CORE + g]["oT"]
        out[b] = acc.T.astype(np.float32) + cvec
    return out


_NC_CACHE = None


def _get_nc():
    global _NC_CACHE
    if _NC_CACHE is None:
        _NC_CACHE = _build_nc()
    return _NC_CACHE


_RUNNER = None


def _get_runner():
    """Reusable jitted PJRT executable (same lowering run_bass_kernel_spmd
    uses under axon) so repeated kernel() calls skip recompilation."""
    global _RUNNER
    if _RUNNER is not None:
        return _RUNNER
    import jax
    from jax.sharding import Mesh, PartitionSpec
    try:
        from jax.experimental.shard_map import shard_map
    except ImportError:
        from jax import shard_map
    from concourse import bass2jax

    nc = _get_nc()
    bass2jax.install_neuronx_cc_hook()
    partition_name = (nc.partition_id_tensor.name
                      if nc.partition_id_tensor else None)
    in_names, out_names, out_avals, zero_outs = [], [], [], []
    for alloc in nc.m.functions[0].allocations:
        if not isinstance(alloc, mybir.MemoryLocationSet):
            continue
        name = alloc.memorylocations[0].name
        if alloc.kind == "ExternalInput":
            if name != partition_name:
                in_names.append(name)
        elif alloc.kind == "ExternalOutput":
            out_names.append(name)
            shape = tuple(alloc.tensor_shape)
            dtype = mybir.dt.np(alloc.dtype)
            out_avals.append(jax.core.ShapedArray(shape, dtype))
            zero_outs.append(np.zeros(shape, dtype))
    n_params = len(in_names)
    all_in = list(in_names) + list(out_names)
    if partition_name is not None:
        all_in.append(partition_name)

    def _b(*args):
        operands = list(args)
        if partition_name is not None:
            operands.append(bass2jax.partition_id_tensor())
        return tuple(bass2jax._bass_exec_p.bind(
            *operands, out_avals=tuple(out_avals), in_names=tuple(all_in),
            out_names=tuple(out_names), lowering_input_output_aliases=(),
            sim_require_finite=True, sim_require_nnan=True, nc=nc))

    devices = jax.devices()[:N_CORES]
    mesh = Mesh(np.asarray(devices), ("core",))
    nio = n_params + len(out_avals)
    fn = jax.jit(shard_map(_b, mesh=mesh,
                           in_specs=(PartitionSpec("core"),) * nio,
                           out_specs=(PartitionSpec("core"),) * len(out_avals),
                           check_rep=False), keep_unused=True)

    def run(in_maps):
        per_core = [[np.asarray(m[n]) for n in in_names] for m in in_maps]
        concat_in = [np.concatenate([per_core[c][i] for c in range(N_CORES)],
                                    axis=0) for i in range(n_params)]
        concat_zeros = [np.zeros((N_CORES * z.shape[0], *z.shape[1:]), z.dtype)
                        for z in zero_outs]
        outs = fn(*concat_in, *concat_zeros)
        return [
            {name: np.asarray(outs[i]).reshape(N_CORES, *out_avals[i].shape)[c]
             for i, name in enumerate(out_names)}
            for c in range(N_CORES)
        ]

    _RUNNER = run
    return run


_CALLED = False


def kernel(**inputs) -> np.ndarray:
    """Full-input entry point: shards across 8 NeuronCores, runs the Bass
    kernel, gathers and unshards. First call uses run_bass_kernel_spmd
    (compile + run); later calls reuse the cached executable."""
    global _CALLED
    in_maps = _make_in_maps(inputs)
    if not _CALLED:
        _CALLED = True
        nc = _get_nc()
        res = run_bass_kernel_spmd(nc, in_maps, list(range(N_CORES)))
        results = res.results
    else:
        results = _get_runner()(in_maps)
    return _assemble(results, inputs)
